# revision 34
# baseline (speedup 1.0000x reference)
"""Trainium2 Bass kernel for DeepgazeSpadeV2 segment_reduce.

Computes, for feats [B=2, C=768, 18, 18] and segmap [B=2, 256, 256] (S=256):
  1. nearest-downsample segmap to 18x18 patch segment ids
  2. scatter-mean patch features into a per-batch [S, C] table
  3. paint: out[b, :, y, x] = table_b[segmap[b, y, x], :]  -> [B, C, 256, 256]

Sharding: 8 cores = 2 batches x 4 row-slices of the output image. Each core
rebuilds its batch's (tiny) segment table and paints its 64-row slice.

On-device algorithm (per core), all-bf16 datapath (rel err ~2e-3 vs the
2e-2 gate; bf16 output also halves HBM write traffic):
  - scatter:  sums[s, c] = onehot_patch[p, s]^T @ featsT[p, c]  (bf16 PE,
              fp32 PSUM accumulate; feats pre-rounded to bf16 on host)
              counts[s]  = onehot_patch^T @ ones   (exact: integer counts)
              table = bf16(sums * (1 / max(counts, 1)))
  - paint:    out[c_tile, pix] = sum_s table[s, c] * onehot_pix[s, pix] as
              bf16 matmuls (FWL, 1 col/cycle: N=512 MM issues every ~216ns
              warm). The one-hot rhs selects bf16 table values bit-exactly.
              PSUM -> bf16 SBUF stage (ACT/DVE convert copies) -> 512KB
              HWDGE DMAs; host upcasts bf16 -> fp32.

Schedule (the previous rev lost ~24us to a serial phase-A prefix):
  - input DMAs ordered segp -> segpix -> feats so the DVE/Pool prep work
    (patch one-hots, segment-id broadcasts) starts as early as possible
  - PE warmup trickle (N=512 accums) trips the HAM clock gate during the
    input-DMA wait so scatter+paint run at 2.4GHz
  - one-hot prep for superblock sb+1 is emitted BEFORE the copies of sb on
    DVE/Pool (software pipelining), so the PE matmul stream never starves
  - paint one-hots split DVE/Pool; PSUM->SBUF copies split ACT/DVE
"""

import sys

if "/opt/trn_rl_repo" not in sys.path:
    sys.path.insert(0, "/opt/trn_rl_repo")

import numpy as np
import ml_dtypes

B, C, HP, WP = 2, 768, 18, 18
HI, WI = 256, 256
S = 256
NP_PATCH = HP * WP            # 324
P_PAD = 384                   # 3 x 128 partition chunks
N_CORES = 8
SLICES_PER_BATCH = N_CORES // B
ROWS_PER_SLICE = HI // SLICES_PER_BATCH   # 64
NPIX = ROWS_PER_SLICE * WI                # 16384
BLK = 512                                 # pixels per PSUM half-block
SUPER = 4                                 # 512-pix blocks per superblock
SBLK = SUPER * BLK                        # 2048 pixels per superblock
NSB = NPIX // SBLK                        # 8 superblocks
CT = C // 128                             # 6 channel tiles
NWARM = 6

_CACHE = {}


def _build():
    import concourse.bacc as bacc
    import concourse.mybir as mybir
    from concourse.tile import TileContext

    f32 = mybir.dt.float32
    bf16 = mybir.dt.bfloat16
    EQ = mybir.AluOpType.is_equal
    MULT = mybir.AluOpType.mult

    nc = bacc.Bacc("TRN2", target_bir_lowering=False, debug=False)
    featsT = nc.dram_tensor("featsT", [128, 3 * C], f32, kind="ExternalInput")
    segp = nc.dram_tensor("segp", [128, 3], f32, kind="ExternalInput")
    # segment ids pre-replicated to all 128 partitions on the host: loading
    # them by plain HWDGE DMA avoids gpsimd partition_broadcast, whose Q7
    # library load costs ~10us at first use and whose execution knocks
    # concurrent DVE tensor ops out of their perf mode (~5x slowdown)
    segrep = nc.dram_tensor("segrep", [128, NPIX], bf16, kind="ExternalInput")
    out = nc.dram_tensor("out", [C, NPIX], bf16, kind="ExternalOutput")

    with TileContext(nc) as tc:
        with (
            tc.tile_pool(name="const", bufs=1) as cp,
            tc.tile_pool(name="work", bufs=3) as wp,
            tc.tile_pool(name="stage", bufs=2) as sp,
        ):
            # ---- input DMAs, latency-critical first ----
            # feats DMAs first: the sum-matmuls gate the paint start, and
            # each DMA pays ~2us completion latency on top of transfer time
            ftk = [
                cp.tile([128, C], f32, tag=f"ft{k}", name=f"ft{k}") for k in range(3)
            ]
            ftr = featsT.ap().rearrange("p (k c) -> p k c", k=3)
            for k in range(3):
                nc.sync.dma_start(out=ftk[k][:, :], in_=ftr[:, k, :])
            sp_f = cp.tile([128, 3], f32, tag="sp_f")
            nc.sync.dma_start(out=sp_f[:, :], in_=segp.ap())

            def load_sgb(sb):
                sgb = wp.tile([128, SBLK], bf16, tag="sgb", bufs=3, name="sgb")
                nc.sync.dma_start(
                    out=sgb[:, :], in_=segrep.ap()[:, sb * SBLK : (sb + 1) * SBLK]
                )
                return sgb

            sgb0 = load_sgb(0)

            # constants (DVE so they land right after the preamble)
            warm_w = cp.tile([128, 64], bf16, tag="warm_w")
            nc.vector.memset(warm_w[:, :], 1.0)
            warm_x = cp.tile([128, BLK], bf16, tag="warm_x")
            nc.vector.memset(warm_x[:, :], 1.0)
            ones_col = cp.tile([128, 1], f32, tag="ones_col")
            nc.vector.memset(ones_col[:, :], 1.0)

            io_f = cp.tile([128, S], f32, tag="io_f")
            nc.gpsimd.iota(io_f[:, :], pattern=[[1, S]], base=0, channel_multiplier=0,
                           allow_small_or_imprecise_dtypes=True)
            io2_f = cp.tile([128, 2], f32, tag="io2_f")
            nc.gpsimd.iota(io2_f[:, :], pattern=[[128, 2]], base=0, channel_multiplier=1,
                           allow_small_or_imprecise_dtypes=True)

            # ---- PE warmup trickle: trip the HAM clock gate during DMA wait
            psW_cm = tc.tile_pool(name="psW", bufs=1, space="PSUM")
            psW = psW_cm.__enter__()
            ps_warm = psW.tile([64, BLK], f32, tag="warm")
            for i in range(NWARM):
                nc.tensor.matmul(
                    ps_warm[:, :], warm_w[:, :], warm_x[:, :],
                    start=(i == 0), stop=(i == NWARM - 1),
                )
            psW_cm.__exit__(None, None, None)

            # one-hot over patches: ohp[p, k, s] = (segp[k*128+p] == s)
            ohp = cp.tile([128, 3, S], f32, tag="ohp")
            for k in range(3):
                nc.vector.tensor_scalar(ohp[:, k, :], io_f[:, :], sp_f[:, k : k + 1], None, EQ)

            # counts / recip per s-tile
            psA_cm = tc.tile_pool(name="psA", bufs=4, space="PSUM")
            psA = psA_cm.__enter__()
            recip = cp.tile([128, 2], f32, tag="recip")
            for st in range(2):
                ps_cnt = psA.tile([128, 1], f32, tag="cnt")
                for k in range(3):
                    nc.tensor.matmul(
                        ps_cnt[:, :],
                        ohp[:, k, st * 128 : (st + 1) * 128],
                        ones_col[:, :],
                        start=(k == 0),
                        stop=(k == 2),
                    )
                cnt_cl = wp.tile([128, 1], f32, tag="cnt_cl")
                nc.vector.tensor_scalar_max(cnt_cl[:, :], ps_cnt[:, :], 1.0)
                nc.vector.reciprocal(recip[:, st : st + 1], cnt_cl[:, :])

            # sums and mean -> per-k-half tables tab0/tab1 (bf16). Split
            # tiles so the first paint LDWEIGHTS (k=0) only waits on tab0.
            tabs = [
                cp.tile([128, C], bf16, tag=f"tab{st}", name=f"tab{st}")
                for st in range(2)
            ]
            for st in range(2):
                for cc in range(2):
                    ps_sum = psA.tile([128, 384], f32, tag="sums")
                    for k in range(3):
                        nc.tensor.matmul(
                            ps_sum[:, :],
                            ohp[:, k, st * 128 : (st + 1) * 128],
                            ftk[k][:, cc * 384 : (cc + 1) * 384],
                            start=(k == 0),
                            stop=(k == 2),
                        )
                    nc.vector.tensor_scalar(
                        tabs[st][:, cc * 384 : (cc + 1) * 384],
                        ps_sum[:, :],
                        recip[:, st : st + 1],
                        None,
                        MULT,
                    )
            psA_cm.__exit__(None, None, None)

            # ---- phase B: paint ----
            def build_oh(sgb):
                """build a superblock's one-hots on DVE from its replicated
                segment-id tile (DVE-only: see segrep note above)."""
                oh = wp.tile([128, 2, SBLK], bf16, tag="oh", bufs=2, name="oh")
                for st in range(2):
                    nc.vector.tensor_scalar(
                        oh[:, st, :],
                        sgb[:, :],
                        io2_f[:, st : st + 1],
                        None,
                        EQ,
                    )
                return oh

            copy_flip = [0]
            oh_cur = build_oh(sgb0)
            with tc.tile_pool(name="psB", bufs=4, space="PSUM") as psB:
                for sb in range(NSB):
                    oh_next = build_oh(load_sgb(sb + 1)) if sb + 1 < NSB else None
                    stages = [
                        sp.tile([128, SBLK], bf16, tag=f"stg{ct}", name=f"stg{ct}")
                        for ct in range(CT)
                    ]
                    for ct in range(CT):
                        for half in range(SUPER // 2):
                            ps_o = psB.tile([128, 2 * BLK], f32, tag="out")
                            for k in range(2):
                                for j in range(2):
                                    jj = half * 2 + j
                                    dst = ps_o[:, j * BLK : (j + 1) * BLK]
                                    nc.tensor.matmul(
                                        dst, tabs[k][:, ct * 128 : (ct + 1) * 128],
                                        oh_cur[:, k, jj * BLK : (jj + 1) * BLK],
                                        start=(k == 0), stop=(k == 1),
                                        skip_group_check=True,
                                    )
                            dst_stage = stages[ct][:, half * 2 * BLK : (half + 1) * 2 * BLK]
                            if copy_flip[0] % 7 < 4:
                                nc.scalar.copy(out=dst_stage, in_=ps_o[:, :])
                            else:
                                nc.vector.tensor_copy(dst_stage, ps_o[:, :])
                            copy_flip[0] += 1
                    if sb < NSB - 1:
                        for ct in range(CT):
                            nc.sync.dma_start(
                                out=out.ap()[
                                    ct * 128 : (ct + 1) * 128,
                                    sb * SBLK : (sb + 1) * SBLK,
                                ],
                                in_=stages[ct][:, :],
                            )
                    else:
                        # last superblock: per-half DMAs so the final (smaller)
                        # transfer starts as soon as its copy lands
                        HB = SBLK // 2
                        for half in range(2):
                            for ct in range(CT):
                                nc.sync.dma_start(
                                    out=out.ap()[
                                        ct * 128 : (ct + 1) * 128,
                                        sb * SBLK + half * HB : sb * SBLK + (half + 1) * HB,
                                    ],
                                    in_=stages[ct][:, half * HB : (half + 1) * HB],
                                )
                    oh_cur = oh_next
    nc.compile()
    return nc


def _get_nc():
    if "nc" not in _CACHE:
        _CACHE["nc"] = _build()
    return _CACHE["nc"]


def _make_in_maps(feats, segmap):
    idx_h = (np.arange(HP) * HI) // HP
    idx_w = (np.arange(WP) * WI) // WP
    in_maps = []
    for core in range(N_CORES):
        b = core // SLICES_PER_BATCH
        q = core % SLICES_PER_BATCH
        ftp = np.zeros((P_PAD, C), dtype=np.float32)
        ftp[:NP_PATCH] = feats[b].reshape(C, NP_PATCH).T
        # [P_PAD, C] -> [128, 3*C] so one contiguous DMA feeds ft[p, k, c]
        ftp = ftp.reshape(3, 128, C).transpose(1, 0, 2).reshape(128, 3 * C)
        spp = np.full((P_PAD,), S, dtype=np.float32)  # pad with S (matches no segment)
        seg_b = np.clip(segmap[b], 0, S - 1)  # reference clips ids to [0, S-1]
        spp[:NP_PATCH] = seg_b[idx_h[:, None], idx_w[None, :]].reshape(-1).astype(np.float32)
        spp = spp.reshape(3, 128).T.copy()  # [128, 3]
        pix = seg_b[q * ROWS_PER_SLICE : (q + 1) * ROWS_PER_SLICE, :].reshape(-1)
        pix_rep = np.ascontiguousarray(
            np.broadcast_to(pix.astype(ml_dtypes.bfloat16), (128, NPIX))
        )
        in_maps.append(
            {
                "featsT": ftp,
                "segp": spp,
                "segrep": pix_rep,
            }
        )
    return in_maps


def _run(in_maps, **kwargs):
    from concourse.bass_utils import run_bass_kernel_spmd

    nc = _get_nc()
    return run_bass_kernel_spmd(nc, in_maps, core_ids=list(range(N_CORES)), **kwargs)


def kernel(feats, segmap, num_total_segments):
    feats = np.asarray(feats, dtype=np.float32)
    segmap = np.asarray(segmap, dtype=np.int32)
    assert int(num_total_segments) == S
    assert feats.shape == (B, C, HP, WP) and segmap.shape == (B, HI, WI)

    res = _run(_make_in_maps(feats, segmap))
    out = np.empty((B, C, HI, WI), dtype=np.float32)
    for core in range(N_CORES):
        b = core // SLICES_PER_BATCH
        q = core % SLICES_PER_BATCH
        out[b, :, q * ROWS_PER_SLICE : (q + 1) * ROWS_PER_SLICE, :] = (
            res.results[core]["out"].astype(np.float32).reshape(C, ROWS_PER_SLICE, WI)
        )
    return out


# revision 40
# speedup vs baseline: 1.0420x; 1.0420x over previous
"""Trainium2 Bass kernel for DeepgazeSpadeV2 segment_reduce.

Computes, for feats [B=2, C=768, 18, 18] and segmap [B=2, 256, 256] (S=256):
  1. nearest-downsample segmap to 18x18 patch segment ids
  2. scatter-mean patch features into a per-batch [S, C] table
  3. paint: out[b, :, y, x] = table_b[segmap[b, y, x], :]  -> [B, C, 256, 256]

Sharding: 8 cores = 2 batches x 4 row-slices of the output image. Each core
rebuilds its batch's (tiny) segment table and paints its 64-row slice.

On-device algorithm (per core), all-bf16 datapath (rel err ~2e-3 vs the
2e-2 gate; bf16 output also halves HBM write traffic):
  - scatter:  sums[s, c] = onehot_patch[p, s]^T @ featsT[p, c]  (bf16 PE,
              fp32 PSUM accumulate; feats pre-rounded to bf16 on host)
              counts[s]  = onehot_patch^T @ ones   (exact: integer counts)
              table = bf16(sums * (1 / max(counts, 1)))
  - paint:    out[c_tile, pix] = sum_s table[s, c] * onehot_pix[s, pix] as
              bf16 matmuls (FWL, 1 col/cycle: N=512 MM issues every ~216ns
              warm). The one-hot rhs selects bf16 table values bit-exactly.
              PSUM -> bf16 SBUF stage (ACT/DVE convert copies) -> 512KB
              HWDGE DMAs; host upcasts bf16 -> fp32.

Schedule (the previous rev lost ~24us to a serial phase-A prefix):
  - input DMAs ordered segp -> segpix -> feats so the DVE/Pool prep work
    (patch one-hots, segment-id broadcasts) starts as early as possible
  - PE warmup trickle (N=512 accums) trips the HAM clock gate during the
    input-DMA wait so scatter+paint run at 2.4GHz
  - one-hot prep for superblock sb+1 is emitted BEFORE the copies of sb on
    DVE/Pool (software pipelining), so the PE matmul stream never starves
  - paint one-hots split DVE/Pool; PSUM->SBUF copies split ACT/DVE
"""

import sys

if "/opt/trn_rl_repo" not in sys.path:
    sys.path.insert(0, "/opt/trn_rl_repo")

import numpy as np
import ml_dtypes

B, C, HP, WP = 2, 768, 18, 18
HI, WI = 256, 256
S = 256
NP_PATCH = HP * WP            # 324
P_PAD = 384                   # 3 x 128 partition chunks
N_CORES = 8
SLICES_PER_BATCH = N_CORES // B
ROWS_PER_SLICE = HI // SLICES_PER_BATCH   # 64
NPIX = ROWS_PER_SLICE * WI                # 16384
BLK = 512                                 # pixels per PSUM half-block
SUPER = 4                                 # 512-pix blocks per superblock
SBLK = SUPER * BLK                        # 2048 pixels per superblock
NSB = NPIX // SBLK                        # 8 superblocks
CT = C // 128                             # 6 channel tiles
NWARM = 10

_CACHE = {}


def _build():
    import concourse.bacc as bacc
    import concourse.mybir as mybir
    from concourse.tile import TileContext

    f32 = mybir.dt.float32
    bf16 = mybir.dt.bfloat16
    EQ = mybir.AluOpType.is_equal
    MULT = mybir.AluOpType.mult

    nc = bacc.Bacc("TRN2", target_bir_lowering=False, debug=False)
    featsT = nc.dram_tensor("featsT", [128, 3 * C], f32, kind="ExternalInput")
    segp = nc.dram_tensor("segp", [128, 3], f32, kind="ExternalInput")
    # segment ids pre-replicated to all 128 partitions on the host: loading
    # them by plain HWDGE DMA avoids gpsimd partition_broadcast, whose Q7
    # library load costs ~10us at first use and whose execution knocks
    # concurrent DVE tensor ops out of their perf mode (~5x slowdown)
    segrep = nc.dram_tensor("segrep", [128, NPIX], bf16, kind="ExternalInput")
    out = nc.dram_tensor("out", [C, NPIX], bf16, kind="ExternalOutput")

    with TileContext(nc) as tc:
        with (
            tc.tile_pool(name="const", bufs=1) as cp,
            tc.tile_pool(name="work", bufs=3) as wp,
            tc.tile_pool(name="stage", bufs=2) as sp,
        ):
            # ---- input DMAs, latency-critical first ----
            sp_f = cp.tile([128, 3], f32, tag="sp_f")
            nc.sync.dma_start(out=sp_f[:, :], in_=segp.ap())

            def load_sgb(sb):
                sgb = wp.tile([128, SBLK], bf16, tag="sgb", bufs=3, name="sgb")
                nc.sync.dma_start(
                    out=sgb[:, :], in_=segrep.ap()[:, sb * SBLK : (sb + 1) * SBLK]
                )
                return sgb

            sgb0 = load_sgb(0)
            # one tile+DMA per k-chunk so the k=0 sum-matmuls can start while
            # the rest of feats is still streaming in
            ftk = [
                cp.tile([128, C], f32, tag=f"ft{k}", name=f"ft{k}") for k in range(3)
            ]
            ftr = featsT.ap().rearrange("p (k c) -> p k c", k=3)
            for k in range(3):
                nc.sync.dma_start(out=ftk[k][:, :], in_=ftr[:, k, :])

            # constants (DVE so they land right after the preamble)
            warm_w = cp.tile([128, 64], bf16, tag="warm_w")
            nc.vector.memset(warm_w[:, :], 1.0)
            warm_x = cp.tile([128, BLK], bf16, tag="warm_x")
            nc.vector.memset(warm_x[:, :], 1.0)
            ones_col = cp.tile([128, 1], f32, tag="ones_col")
            nc.vector.memset(ones_col[:, :], 1.0)

            io_f = cp.tile([128, S], f32, tag="io_f")
            nc.gpsimd.iota(io_f[:, :], pattern=[[1, S]], base=0, channel_multiplier=0,
                           allow_small_or_imprecise_dtypes=True)
            io2_f = cp.tile([128, 2], f32, tag="io2_f")
            nc.gpsimd.iota(io2_f[:, :], pattern=[[128, 2]], base=0, channel_multiplier=1,
                           allow_small_or_imprecise_dtypes=True)

            # ---- PE warmup trickle: trip the HAM clock gate during DMA wait
            psW_cm = tc.tile_pool(name="psW", bufs=1, space="PSUM")
            psW = psW_cm.__enter__()
            ps_warm = psW.tile([64, BLK], f32, tag="warm")
            for i in range(NWARM):
                nc.tensor.matmul(
                    ps_warm[:, :], warm_w[:, :], warm_x[:, :],
                    start=(i == 0), stop=(i == NWARM - 1),
                )
            psW_cm.__exit__(None, None, None)

            # one-hot over patches: ohp[p, k, s] = (segp[k*128+p] == s)
            ohp = cp.tile([128, 3, S], f32, tag="ohp")
            for k in range(3):
                nc.vector.tensor_scalar(ohp[:, k, :], io_f[:, :], sp_f[:, k : k + 1], None, EQ)

            # counts / recip per s-tile
            psA_cm = tc.tile_pool(name="psA", bufs=4, space="PSUM")
            psA = psA_cm.__enter__()
            recip = cp.tile([128, 2], f32, tag="recip")
            for st in range(2):
                ps_cnt = psA.tile([128, 1], f32, tag="cnt")
                for k in range(3):
                    nc.tensor.matmul(
                        ps_cnt[:, :],
                        ohp[:, k, st * 128 : (st + 1) * 128],
                        ones_col[:, :],
                        start=(k == 0),
                        stop=(k == 2),
                    )
                cnt_cl = wp.tile([128, 1], f32, tag="cnt_cl")
                nc.vector.tensor_scalar_max(cnt_cl[:, :], ps_cnt[:, :], 1.0)
                nc.vector.reciprocal(recip[:, st : st + 1], cnt_cl[:, :])

            # sums and mean -> table [s%128, s//128, c] (bf16)
            tab = cp.tile([128, 2, C], bf16, tag="tab")
            for st in range(2):
                for cc in range(2):
                    ps_sum = psA.tile([128, 384], f32, tag="sums")
                    for k in range(3):
                        nc.tensor.matmul(
                            ps_sum[:, :],
                            ohp[:, k, st * 128 : (st + 1) * 128],
                            ftk[k][:, cc * 384 : (cc + 1) * 384],
                            start=(k == 0),
                            stop=(k == 2),
                        )
                    nc.vector.tensor_scalar(
                        tab[:, st, cc * 384 : (cc + 1) * 384],
                        ps_sum[:, :],
                        recip[:, st : st + 1],
                        None,
                        MULT,
                    )
            psA_cm.__exit__(None, None, None)

            # ---- phase B: paint ----
            def build_oh(sgb):
                """build a superblock's one-hots on DVE from its replicated
                segment-id tile (DVE-only: see segrep note above)."""
                oh = wp.tile([128, 2, SBLK], bf16, tag="oh", bufs=2, name="oh")
                for st in range(2):
                    nc.vector.tensor_scalar(
                        oh[:, st, :],
                        sgb[:, :],
                        io2_f[:, st : st + 1],
                        None,
                        EQ,
                    )
                return oh

            copy_flip = [0]
            oh_cur = build_oh(sgb0)
            with tc.tile_pool(name="psB", bufs=4, space="PSUM") as psB:
                for sb in range(NSB):
                    oh_next = build_oh(load_sgb(sb + 1)) if sb + 1 < NSB else None
                    stages = [
                        sp.tile([128, SBLK], bf16, tag=f"stg{ct}", name=f"stg{ct}")
                        for ct in range(CT)
                    ]
                    for ct in range(CT):
                        for half in range(SUPER // 2):
                            ps_o = psB.tile([128, 2 * BLK], f32, tag="out")
                            for k in range(2):
                                for j in range(2):
                                    jj = half * 2 + j
                                    dst = ps_o[:, j * BLK : (j + 1) * BLK]
                                    nc.tensor.matmul(
                                        dst, tab[:, k, ct * 128 : (ct + 1) * 128],
                                        oh_cur[:, k, jj * BLK : (jj + 1) * BLK],
                                        start=(k == 0), stop=(k == 1),
                                        skip_group_check=True,
                                    )
                            dst_stage = stages[ct][:, half * 2 * BLK : (half + 1) * 2 * BLK]
                            if copy_flip[0] % 7 < 4:
                                nc.scalar.copy(out=dst_stage, in_=ps_o[:, :])
                            else:
                                nc.vector.tensor_copy(dst_stage, ps_o[:, :])
                            copy_flip[0] += 1
                    for ct in range(CT):
                        nc.sync.dma_start(
                            out=out.ap()[
                                ct * 128 : (ct + 1) * 128,
                                sb * SBLK : (sb + 1) * SBLK,
                            ],
                            in_=stages[ct][:, :],
                        )
                    oh_cur = oh_next
    nc.compile()
    return nc


def _get_nc():
    if "nc" not in _CACHE:
        _CACHE["nc"] = _build()
    return _CACHE["nc"]


def _make_in_maps(feats, segmap):
    idx_h = (np.arange(HP) * HI) // HP
    idx_w = (np.arange(WP) * WI) // WP
    in_maps = []
    for core in range(N_CORES):
        b = core // SLICES_PER_BATCH
        q = core % SLICES_PER_BATCH
        ftp = np.zeros((P_PAD, C), dtype=np.float32)
        ftp[:NP_PATCH] = feats[b].reshape(C, NP_PATCH).T
        # [P_PAD, C] -> [128, 3*C] so one contiguous DMA feeds ft[p, k, c]
        ftp = ftp.reshape(3, 128, C).transpose(1, 0, 2).reshape(128, 3 * C)
        spp = np.full((P_PAD,), S, dtype=np.float32)  # pad with S (matches no segment)
        seg_b = np.clip(segmap[b], 0, S - 1)  # reference clips ids to [0, S-1]
        spp[:NP_PATCH] = seg_b[idx_h[:, None], idx_w[None, :]].reshape(-1).astype(np.float32)
        spp = spp.reshape(3, 128).T.copy()  # [128, 3]
        pix = seg_b[q * ROWS_PER_SLICE : (q + 1) * ROWS_PER_SLICE, :].reshape(-1)
        pix_rep = np.ascontiguousarray(
            np.broadcast_to(pix.astype(ml_dtypes.bfloat16), (128, NPIX))
        )
        in_maps.append(
            {
                "featsT": ftp,
                "segp": spp,
                "segrep": pix_rep,
            }
        )
    return in_maps


def _run(in_maps, **kwargs):
    from concourse.bass_utils import run_bass_kernel_spmd

    nc = _get_nc()
    return run_bass_kernel_spmd(nc, in_maps, core_ids=list(range(N_CORES)), **kwargs)


def kernel(feats, segmap, num_total_segments):
    feats = np.asarray(feats, dtype=np.float32)
    segmap = np.asarray(segmap, dtype=np.int32)
    assert int(num_total_segments) == S
    assert feats.shape == (B, C, HP, WP) and segmap.shape == (B, HI, WI)

    res = _run(_make_in_maps(feats, segmap))
    out = np.empty((B, C, HI, WI), dtype=np.float32)
    for core in range(N_CORES):
        b = core // SLICES_PER_BATCH
        q = core % SLICES_PER_BATCH
        out[b, :, q * ROWS_PER_SLICE : (q + 1) * ROWS_PER_SLICE, :] = (
            res.results[core]["out"].astype(np.float32).reshape(C, ROWS_PER_SLICE, WI)
        )
    return out
